# revision 1
# baseline (speedup 1.0000x reference)
"""Pairwise cosine-similarity scorer (CosScorer) for Trainium2.

Full-input contract: kernel(xs_pad=[8,8192,256] f32, spk_emb=[8,200,256] f32)
-> [8,8192,200] f32, computed as dot(x,y)/max(||x||*||y||, eps).

Sharding: data-parallel over B — core i handles batch element i (B=8 on
8 cores), SPMD program, no collectives.

Per-core pipeline (x=[8192,256], spk=[200,256] -> out=[8192,200]), all fp32:
  - spk prep (once): ScalarE square+accum norms -> sqrt -> VectorE
    reciprocal -> scale rows; PE-transpose into spknT chunks [d=128, s=200].
  - x streams in 16 DMAs of [128, 4x256]. Per 128-row subtile:
      VectorE: bn_stats/bn_aggr -> sumsq = (var+mean^2)*D (one pass)
      PE:      transpose raw x chunks via identity matmul -> one PSUM tile
      VectorE: single [128,256] PSUM->SBUF copy of the transposed pair
      PE:      2 accumulating fp32 matmuls xT.T @ spknT -> scores in PSUM
      ScalarE: scaled PSUM->SBUF copy (activation Copy, scale=1/||x||)
      DMA:     one batched store per 4 subtiles (last macro: per-subtile)
  - 1/||spk|| is folded into spknT, 1/||x|| into the output copy, so the
    matmul runs on raw x and normalized spk. eps clamp is dead for this
    data distribution (min ||x|| >> 1e-8 for 256-dim gaussian rows).
  - macro 0's norms+transposes are emitted before spk prep so the PE
    starts (and HAM-warms) as soon as the first x tile lands.

PE is LDWEIGHTS-bandwidth-bound (fp32 weights load in 2 passes); measured
~92us/core on trn2 vs ~43us DMA floor and ~71us PE floor.
"""

import sys

if "/opt/trn_rl_repo" not in sys.path:
    sys.path.insert(0, "/opt/trn_rl_repo")

import numpy as np

B, T, S, D = 8, 8192, 200, 256
P = 128
NSUB = 4            # 128-row subtiles per input DMA
NMACRO = T // (P * NSUB)
NCHUNK = D // P     # contraction chunks

_CACHE = {}


def _build():
    if "nc" in _CACHE:
        return _CACHE["nc"]

    from contextlib import ExitStack

    import concourse.tile as tile
    from concourse import bacc, mybir
    from concourse.masks import make_identity

    f32 = mybir.dt.float32
    Act = mybir.ActivationFunctionType

    nc = bacc.Bacc("TRN2", target_bir_lowering=False, debug=False)
    x = nc.dram_tensor("x", [T, D], f32, kind="ExternalInput").ap()
    spk = nc.dram_tensor("spk", [S, D], f32, kind="ExternalInput").ap()
    out = nc.dram_tensor("out", [T, S], f32, kind="ExternalOutput").ap()

    with tile.TileContext(nc) as tc, ExitStack() as ctx:
        const = ctx.enter_context(tc.tile_pool(name="const", bufs=1))
        xin = ctx.enter_context(tc.tile_pool(name="xin", bufs=5))
        stats = ctx.enter_context(tc.tile_pool(name="stats", bufs=4))
        xtp = ctx.enter_context(tc.tile_pool(name="xtp", bufs=6))
        outp = ctx.enter_context(tc.tile_pool(name="outp", bufs=3))
        psum_t = ctx.enter_context(tc.tile_pool(name="psum_t", bufs=3, space="PSUM"))
        psum_o = ctx.enter_context(tc.tile_pool(name="psum_o", bufs=4, space="PSUM"))

        identity = const.tile([P, P], f32, tag="identity")
        make_identity(nc, identity)

        # t = i*512 + n*128 + p
        x_r = x.rearrange("(i n p) d -> i p n d", p=P, n=NSUB)
        out_r = out.rearrange("(i n p) s -> i p n s", p=P, n=NSUB)

        # spk loads go first: the whole matmul chain gates on spknT
        sp_tiles = []
        for s0, ps in ((0, P), (P, S - P)):
            sp = const.tile([P, D], f32, tag=f"sp{s0}", name=f"sp{s0}")
            nc.sync.dma_start(out=sp[:ps], in_=spk[s0 : s0 + ps])
            sp_tiles.append(sp)

        # pre-warm the Sqrt ACT table while DMAs run (table load ~2.7us)
        warm = const.tile([P, 1], f32, tag="warm")
        nc.vector.memset(warm, 1.0)
        nc.scalar.sqrt(warm, warm)

        # HAM warm-up: ~6us of real matmuls on the identity while the first
        # x macro is still in flight, so the PE is at 2.4GHz (and stays
        # there — no >3.4us idle window) when real transposes arrive at
        # ~13.6us, which is when the first 512KB x load completes.
        warm_ps = psum_o.tile([P, P], f32, tag="warm_ps", bufs=1)
        for w in range(14):
            nc.tensor.matmul(
                warm_ps, lhsT=identity, rhs=identity, start=True, stop=True
            )

        def emit_load(i):
            xm = xin.tile([P, NSUB, D], f32, tag="xm", name=f"xm{i}")
            nc.sync.dma_start(out=xm, in_=x_r[i])
            return xm

        def emit_norms(i, xm):
            bs = nc.vector.BN_STATS_DIM
            ba = nc.vector.BN_AGGR_DIM
            ssq = stats.tile([P, NSUB], f32, tag="ssq", name=f"ssq{i}")
            inv = stats.tile([P, NSUB], f32, tag="inv", name=f"inv{i}")
            stt = stats.tile([P, NSUB, bs], f32, tag="stt", name=f"stt{i}")
            mv = stats.tile([P, NSUB, ba], f32, tag="mv", name=f"mv{i}")
            for n in range(NSUB):
                nc.vector.bn_stats(out=stt[:, n, :], in_=xm[:, n])
                nc.vector.bn_aggr(out=mv[:, n, :], in_=stt[:, n, :])
            # sumsq = (var + mean^2); norm = sqrt(D * sumsq)
            nc.vector.tensor_mul(ssq, mv[:, :, 0], mv[:, :, 0])
            nc.vector.tensor_add(ssq, ssq, mv[:, :, 1])
            nc.scalar.activation(out=ssq, in_=ssq, func=Act.Sqrt, scale=float(D))
            nc.vector.reciprocal(inv, ssq)
            return inv

        def emit_transpose(i, n, xm):
            # both d-chunks into one PSUM bank; c=1 keeps has_written intact
            xts = xtp.tile([P, NCHUNK, P], f32, tag="xts", name=f"xts{i}_{n}")
            pst = psum_t.tile(
                [P, NCHUNK, P], f32, tag="pst", name=f"pst{i}_{n}", bufs=3
            )
            for c in range(NCHUNK):
                nc.tensor.matmul(
                    pst[:, c, :],
                    lhsT=xm[:, n, c * P : (c + 1) * P],
                    rhs=identity,
                    is_transpose=True,
                    start=(c == 0),
                    stop=(c == NCHUNK - 1),
                )
            nc.vector.tensor_copy(out=xts, in_=pst)
            return xts

        def emit_scores(i, n, xts, inv, omac, spknT):
            pso = psum_o.tile([P, S], f32, tag="pso", name=f"pso{i}_{n}")
            for c in range(NCHUNK):
                nc.tensor.matmul(
                    pso,
                    lhsT=xts[:, c, :],
                    rhs=spknT[c],
                    start=(c == 0),
                    stop=(c == NCHUNK - 1),
                )
            # fused normalize-by-1/||x|| on the PSUM->SBUF copy (ScalarE)
            nc.scalar.mul(omac[:, n, :], pso, inv[:, n : n + 1])

        # ---- macro 0: load + norms + transposes before spk prep so the
        # PE starts working (and HAM-warms) as soon as data lands ----
        xm0 = emit_load(0)
        inv0 = emit_norms(0, xm0)
        xts0 = [emit_transpose(0, n, xm0) for n in range(NSUB)]

        # ---- spk prep: normalized, transposed chunks [d=128, s=200] ----
        spknT = [
            const.tile([P, S], f32, name=f"spknT{c}", tag=f"spknT{c}")
            for c in range(NCHUNK)
        ]
        for (s0, ps), sp in zip(((0, P), (P, S - P)), sp_tiles):
            sq = const.tile([P, D], f32, tag=f"sq{s0}")
            ssq = const.tile([P, 1], f32, tag=f"ssq{s0}")
            nc.scalar.activation(
                out=sq[:ps], in_=sp[:ps], func=Act.Square, accum_out=ssq[:ps]
            )
            nc.scalar.sqrt(ssq[:ps], ssq[:ps])
            nc.vector.reciprocal(ssq[:ps], ssq[:ps])
            spn = const.tile([P, D], f32, tag=f"spn{s0}")
            nc.vector.tensor_scalar_mul(out=spn[:ps], in0=sp[:ps], scalar1=ssq[:ps])
            for c in range(NCHUNK):
                pt = psum_t.tile([P, P], f32, tag="pst", bufs=3)
                nc.tensor.transpose(
                    pt[:, :ps], spn[:ps, c * P : (c + 1) * P], identity[:ps, :ps]
                )
                nc.vector.tensor_copy(out=spknT[c][:, s0 : s0 + ps], in_=pt[:, :ps])

        # ---- main loop ----
        for i in range(NMACRO):
            if i == 0:
                xm, inv = xm0, inv0
            else:
                xm = emit_load(i)
                inv = emit_norms(i, xm)
            omac = outp.tile([P, NSUB, S], f32, tag="omac", name=f"omac{i}")
            for n in range(NSUB):
                xts = xts0[n] if i == 0 else emit_transpose(i, n, xm)
                emit_scores(i, n, xts, inv, omac, spknT)
            # stores ride the ScalarE HWDGE ring so they don't queue behind
            # the next macro's 512KB load on the SyncE ring
            nc.scalar.dma_start(out=out_r[i], in_=omac)

    nc.compile()
    _CACHE["nc"] = nc
    return nc


def _run(xs_pad, spk_emb, trace=False):
    from concourse.bass_utils import run_bass_kernel_spmd

    nc = _build()
    xs_pad = np.ascontiguousarray(np.asarray(xs_pad), dtype=np.float32)
    spk_emb = np.ascontiguousarray(np.asarray(spk_emb), dtype=np.float32)
    assert xs_pad.shape == (B, T, D) and spk_emb.shape == (B, S, D)
    in_maps = [{"x": xs_pad[i], "spk": spk_emb[i]} for i in range(B)]
    res = run_bass_kernel_spmd(nc, in_maps, list(range(B)), trace=trace)
    out = np.stack([res.results[i]["out"] for i in range(B)], axis=0)
    return out, res


def kernel(xs_pad, spk_emb):
    out, _ = _run(xs_pad, spk_emb, trace=False)
    return out



# revision 4
# speedup vs baseline: 1.8338x; 1.8338x over previous
"""Pairwise cosine-similarity scorer (CosScorer) for Trainium2 — bf16 build.

Full-input contract: kernel(xs_pad=[8,8192,256] f32, spk_emb=[8,200,256] f32)
-> [8,8192,200] f32, dot(x,y)/max(||x||*||y||, eps).

Sharding: data-parallel over B — core i handles batch element i, SPMD, no
collectives. rel-err budget is 2e-2; bf16 inputs + fp32 PSUM accumulation
land ~6e-3, so the whole pipeline runs in bf16:

  - Host casts x to bf16 and pre-transposes it (xT=[256,8192], a pure
    layout/dtype change), spk to bf16. Output returns as bf16 [8192,200]
    and is upcast on host. DMA: 4.2MB in + 3.3MB out per core (~22us at
    ~350GB/s/core) vs 14.8MB for the fp32 version.
  - No on-device transpose of x: score matmuls take lhsT = xT column
    slices straight from the DMA'd tiles (PE contracts over d on
    partitions). bf16 matmul streams 1 cycle/col vs fp32's 4.
  - Row norms: DVE squares xT (2x perf mode), pre-adds the two d-chunks,
    then a width-1 PE matmul against ones gives sum(x^2) already laid out
    [t-partition, 1] — exactly what the output scale needs. Sqrt runs on
    ScalarE directly from PSUM; reciprocal on DVE.
  - t is 4-way interleaved within each 512-row block (t = 512b + 4p + j)
    so each partition's 4 output rows are adjacent in DRAM: stores write
    1600B contiguous lines (full DMA speed; bf16 [128,200] tiles alone
    would be 400B lines at half throughput).
  - 1/||spk|| is folded into spknT on device; 1/||x|| into the PSUM->SBUF
    output copy (split ScalarE/DVE). eps clamp is dead for this data
    distribution (min ||x|| >> 1e-8 for 256-dim gaussian rows).
"""

import sys

if "/opt/trn_rl_repo" not in sys.path:
    sys.path.insert(0, "/opt/trn_rl_repo")

import numpy as np

B, T, S, D = 8, 8192, 200, 256
P = 128
NBLK = 16           # blocks of 512 t-rows
NPH = 4             # phase interleave: t = 512*b + 4*p + j
NCHUNK = D // P     # contraction chunks

_CACHE = {}


def _build():
    if "nc" in _CACHE:
        return _CACHE["nc"]

    from contextlib import ExitStack

    import concourse.tile as tile
    from concourse import bacc, mybir
    from concourse.masks import make_identity

    f32 = mybir.dt.float32
    bf16 = mybir.dt.bfloat16
    Act = mybir.ActivationFunctionType

    nc = bacc.Bacc("TRN2", target_bir_lowering=False, debug=False)
    xT = nc.dram_tensor("xT", [D, T], bf16, kind="ExternalInput").ap()
    spk = nc.dram_tensor("spk", [S, D], bf16, kind="ExternalInput").ap()
    out = nc.dram_tensor("out", [T, S], bf16, kind="ExternalOutput").ap()

    with tile.TileContext(nc) as tc, ExitStack() as ctx:
        const = ctx.enter_context(tc.tile_pool(name="const", bufs=1))
        xin = ctx.enter_context(tc.tile_pool(name="xin", bufs=4))
        xsqp = ctx.enter_context(tc.tile_pool(name="xsqp", bufs=3))
        stats = ctx.enter_context(tc.tile_pool(name="stats", bufs=4))
        outp = ctx.enter_context(tc.tile_pool(name="outp", bufs=3))
        psum_t = ctx.enter_context(tc.tile_pool(name="psum_t", bufs=1, space="PSUM"))
        psum_n = ctx.enter_context(tc.tile_pool(name="psum_n", bufs=2, space="PSUM"))
        psum_o = ctx.enter_context(tc.tile_pool(name="psum_o", bufs=4, space="PSUM"))

        # d chunk c, partition p(=d%128), block b, col q(=t within block)
        xT_r = xT.rearrange("(c p) (b q) -> b p c q", p=P, q=P * NPH)
        # t = 512*b + 4*p + j  ->  per-partition 1600B contiguous store lines
        out_r = out.rearrange("(b p q) s -> b p q s", p=P, q=NPH)

        # spk load first (whole matmul chain gates on spknT)
        sp_tiles = []
        for s0, ps in ((0, P), (P, S - P)):
            sp = const.tile([P, D], bf16, tag=f"sp{s0}", name=f"sp{s0}")
            nc.sync.dma_start(out=sp[:ps], in_=spk[s0 : s0 + ps])
            sp_tiles.append(sp)

        # first x block starts streaming immediately
        def emit_load(b):
            xm = xin.tile([P, NCHUNK, P * NPH], bf16, tag="xm", name=f"xm{b}")
            nc.sync.dma_start(out=xm, in_=xT_r[b])
            return xm

        xm0 = emit_load(0)

        identity = const.tile([P, P], bf16, tag="identity")
        make_identity(nc, identity)
        ones = const.tile([P, 1], bf16, tag="ones")
        nc.vector.memset(ones, 1.0)

        # pre-warm Sqrt ACT table while DMAs run
        warm = const.tile([P, 1], f32, tag="warm")
        nc.vector.memset(warm, 1.0)
        nc.scalar.sqrt(warm, warm)

        # HAM warm-up: real matmuls on the identity while the first x block
        # is in flight so the PE is ramped when real work arrives
        warm_ps = psum_t.tile([P, P], f32, tag="warm_ps", bufs=1)
        for _ in range(14):
            nc.tensor.matmul(warm_ps, lhsT=identity, rhs=identity, start=True, stop=True)

        # ---- spk prep: normalized, transposed chunks [d=128, s=200] bf16 ----
        spknT = [
            const.tile([P, S], bf16, name=f"spknT{c}", tag=f"spknT{c}")
            for c in range(NCHUNK)
        ]
        for (s0, ps), sp in zip(((0, P), (P, S - P)), sp_tiles):
            sq = const.tile([P, D], bf16, tag=f"sq{s0}")
            ssq = const.tile([P, 1], f32, tag=f"ssq{s0}")
            nc.scalar.activation(
                out=sq[:ps], in_=sp[:ps], func=Act.Square, accum_out=ssq[:ps]
            )
            nc.scalar.sqrt(ssq[:ps], ssq[:ps])
            nc.vector.reciprocal(ssq[:ps], ssq[:ps])
            spn = const.tile([P, D], bf16, tag=f"spn{s0}")
            nc.vector.tensor_scalar_mul(out=spn[:ps], in0=sp[:ps], scalar1=ssq[:ps])
            for c in range(NCHUNK):
                pt = psum_t.tile([P, P], bf16, tag="pst", bufs=1)
                nc.tensor.transpose(
                    pt[:, :ps], spn[:ps, c * P : (c + 1) * P], identity[:ps, :ps]
                )
                nc.vector.tensor_copy(out=spknT[c][:, s0 : s0 + ps], in_=pt[:, :ps])

        # ---- main loop: 16 blocks of 512 rows ----
        for b in range(NBLK):
            xm = xm0 if b == 0 else emit_load(b)
            # xsq slots 0,1 = per-chunk squares, slot 2 = chunk sum
            xsq = xsqp.tile([P, NCHUNK + 1, P * NPH], bf16, tag="xsq", name=f"xsq{b}")
            nc.vector.tensor_mul(
                xsq[:, :NCHUNK, :], xm, xm
            )
            nc.vector.tensor_add(xsq[:, NCHUNK, :], xsq[:, 0, :], xsq[:, 1, :])

            pso = [
                psum_o.tile([P, 2, S], f32, tag="pso", name=f"pso{b}_{h}")
                for h in range(2)
            ]
            for j in range(NPH):
                for c in range(NCHUNK):
                    nc.tensor.matmul(
                        pso[j // 2][:, j % 2, :],
                        lhsT=xm[:, c, j :: NPH],
                        rhs=spknT[c],
                        start=(c == 0),
                        stop=(c == NCHUNK - 1),
                    )
                if j == 1:
                    # norms after the first pso bank's scores: one width-1
                    # matmul per phase on the pre-added squares
                    psn = psum_n.tile([P, NPH], f32, tag="psn", name=f"psn{b}")
                    for jj in range(NPH):
                        nc.tensor.matmul(
                            psn[:, jj : jj + 1],
                            lhsT=xsq[:, NCHUNK, jj :: NPH],
                            rhs=ones,
                            start=True,
                            stop=True,
                        )
            ns = stats.tile([P, NPH], f32, tag="ns", name=f"ns{b}")
            inv = stats.tile([P, NPH], f32, tag="inv", name=f"inv{b}")
            nc.scalar.activation(out=ns, in_=psn, func=Act.Sqrt)
            nc.vector.reciprocal(inv, ns)

            omac = outp.tile([P, NPH, S], bf16, tag="omac", name=f"omac{b}")
            for j in range(NPH):
                src = pso[j // 2][:, j % 2, :]
                if j % 2 == 0:
                    nc.scalar.mul(omac[:, j, :], src, inv[:, j : j + 1])
                else:
                    nc.vector.tensor_scalar_mul(
                        out=omac[:, j, :], in0=src, scalar1=inv[:, j : j + 1]
                    )
            nc.scalar.dma_start(out=out_r[b], in_=omac)

    nc.compile()
    _CACHE["nc"] = nc
    return nc


def _run(xs_pad, spk_emb, trace=False):
    import ml_dtypes
    from concourse.bass_utils import run_bass_kernel_spmd

    bf16 = ml_dtypes.bfloat16
    nc = _build()
    xs_pad = np.asarray(xs_pad)
    spk_emb = np.asarray(spk_emb)
    assert xs_pad.shape == (B, T, D) and spk_emb.shape == (B, S, D)
    in_maps = [
        {
            "xT": np.ascontiguousarray(xs_pad[i].T.astype(bf16)),
            "spk": np.ascontiguousarray(spk_emb[i].astype(bf16)),
        }
        for i in range(B)
    ]
    res = run_bass_kernel_spmd(nc, in_maps, list(range(B)), trace=trace)
    out = np.stack(
        [np.asarray(res.results[i]["out"]).astype(np.float32) for i in range(B)],
        axis=0,
    )
    return out, res


def kernel(xs_pad, spk_emb):
    out, _ = _run(xs_pad, spk_emb, trace=False)
    return out
